# revision 21
# baseline (speedup 1.0000x reference)
"""CRF negative log-likelihood (sum) on 8 Trainium2 NeuronCores.

Strategy (per core, batch-sharded 1024 -> 8 x 128):
  partition function: linear-space bidirectional scan.
    alpha recurrence rewritten as f_s = G[s] * (E^T f_{s-1}),
    beta  recurrence rewritten as h_t = G[t] * (E h_{t+1}),
    with E = exp(transitions), G[t] = exp(em[t] - delta)  (delta = host-estimated
    mean log-growth, for fp range safety; periodic renormalization adds the
    removed log-factors back).
    Both chains run in the SAME instructions: state S = [f ; h] (96 x batch),
    one blockdiag(E, E^T) matmul + one elementwise multiply per paired step.
    The 128-batch shard is further split into THREE independent ~43-wide
    chains so the serial matmul->multiply dependency chains interleave on
    TensorE/DVE (each engine works on one chain while the others' results are
    in flight), cutting the per-step critical path well below the fused form.
    Z_b = sum_i f_255[i,b] * (E h_256)[i,b];  logZ + 512*delta + C = partition.
  score: no gathers.  The emission part sum_s em[s,b,tag] is a masked sum
    of the batch-major emissions against a host-built one-hot V (0/1 bf16):
    the multiply runs on GPSIMD (hidden under the scan), the summation is a
    short 4x-mode DVE reduce tail.  The transition/start/end part is
    sum_ij C_b[i,j]*T[i,j] with C_b a host-built per-batch pair-count matrix
    (pure tag statistics): 19 PSUM-accumulated matmuls during the load phase.
  mask input is all-ones per the problem spec and is not used.

Layouts (host prepares pure reorderings of the inputs, no arithmetic):
  emT_d [112, 256*128]: tag-major paired emissions; emT[j, k*128+b] = em[k,b,j]
    for j<48, em[511-k,b,j-64] for 64<=j<112, zeros in the 48:64 gap (the gap
    keeps every compute-engine partition base 32-aligned).  DMA'd straight
    into SBUF with an f32->bf16 casting DMA (no on-device transposes), exp'd
    in place.
  emg_d [128, 512*48]: batch-major emissions for the score mask-sum.
"""

import os
from contextlib import ExitStack

import numpy as np

import concourse.bass as bass
import concourse.bacc as bacc
import concourse.mybir as mybir
import concourse.tile as tile
from concourse.bass_utils import run_bass_kernel_spmd

S, B, T = 512, 1024, 48
NCORES = 8
BL = B // NCORES          # 128 batch per core
CH = ((0, 44), (44, 42), (86, 42))  # (offset, width) of the 3 batch chains
NBLK = S // 2             # 256 paired blocks
EMCOLS = NBLK * BL        # 32768 cols in emT
NWIN = 16                 # load/exp windows
WBLK = NBLK // NWIN       # 32 blocks per window
GCOLS = S * T             # 24576 score-mask cols for emissions
NCH = 19                  # 19 x 128 = 2432 padded count rows (2400 used)
CC = NCH * 128            # 2432

f32 = mybir.dt.float32
bf16 = mybir.dt.bfloat16
u16 = mybir.dt.uint16
ALU = mybir.AluOpType
ACT = mybir.ActivationFunctionType

_LAST = {}


def _estimate_delta(em, tr, st, nsamp=16):
    """Exact log-space forward scan on a few batch columns: mean per-step
    log-growth (delta) and renorm period R keeping |log f| bounded."""
    e = np.asarray(em[:, :nsamp, :], dtype=np.float64)
    trr = np.asarray(tr, dtype=np.float64)
    a = np.asarray(st, dtype=np.float64)[None, :] + e[0]
    means = [a.mean(axis=1)]
    for t in range(1, S):
        z = a[:, :, None] + trr[None, :, :]
        m = z.max(axis=1)
        a = e[t] + m + np.log(np.exp(z - m[:, None, :]).sum(axis=1))
        means.append(a.mean(axis=1))
    meanarr = np.stack(means)               # (S, nsamp)
    inc = np.diff(meanarr, axis=0)          # (S-1, nsamp)
    delta = float(inc.mean())
    dev = np.concatenate(
        [np.zeros((1, nsamp)), np.cumsum(inc - delta, axis=0)], axis=0
    )
    R = 16
    for cand in (64, 32, 16):
        wmax = 0.0
        for w0 in range(0, S - 1, cand):
            seg = dev[w0 : w0 + cand + 1] - dev[w0]
            wmax = max(wmax, float(np.abs(seg).max()))
        if 1.5 * wmax + 16.0 < 70.0:
            R = cand
            break
    return delta, R


def _build(delta, renorm_iters):
    nc = bacc.Bacc("TRN2", target_bir_lowering=False, debug=False)
    emT_d = nc.dram_tensor("emT", [112, EMCOLS], f32, kind="ExternalInput")
    emg_d = nc.dram_tensor("emg", [BL, GCOLS], f32, kind="ExternalInput")
    v_d = nc.dram_tensor("vhot", [BL, GCOLS], bf16, kind="ExternalInput")
    c_d = nc.dram_tensor("cmat", [128, CC], bf16, kind="ExternalInput")
    t19_d = nc.dram_tensor("t19", [128, NCH], f32, kind="ExternalInput")
    tr_d = nc.dram_tensor("transitions", [T, T], f32, kind="ExternalInput")
    trT_d = nc.dram_tensor("transitionsT", [T, T], f32, kind="ExternalInput")
    se_d = nc.dram_tensor("startend", [T, 2], f32, kind="ExternalInput")
    out_d = nc.dram_tensor("nll", [BL, 1], f32, kind="ExternalOutput")

    with tile.TileContext(nc) as tc, ExitStack() as ctx:
        big = ctx.enter_context(tc.tile_pool(name="big", bufs=1))
        small = ctx.enter_context(tc.tile_pool(name="small", bufs=1))
        rn_pool = ctx.enter_context(tc.tile_pool(name="rnp", bufs=2))
        psR = ctx.enter_context(tc.tile_pool(name="psR", bufs=3, space="PSUM"))
        psN = ctx.enter_context(tc.tile_pool(name="psN", bufs=1, space="PSUM"))
        psX = ctx.enter_context(tc.tile_pool(name="psX", bufs=1, space="PSUM"))
        ps1 = ctx.enter_context(tc.tile_pool(name="ps1", bufs=1, space="PSUM"))

        emT = big.tile([112, EMCOLS], bf16)
        emg = big.tile([BL, GCOLS], bf16)
        vhot = big.tile([BL, GCOLS], bf16)
        c_sb = big.tile([128, CC], bf16)

        # ---- small constant tiles ----
        tr_sb = small.tile([T, T], f32)
        trT_sb = small.tile([T, T], f32)
        se_sb = small.tile([T, 2], f32)
        es_ee = small.tile([T, 2], f32)
        t19 = small.tile([128, NCH], f32)
        t19b = small.tile([128, NCH], bf16)
        lhs96 = small.tile([112, 112], bf16)
        onesel = small.tile([112, 112], bf16)
        ones48 = small.tile([T, 1], bf16)
        id1 = small.tile([1, 1], f32)
        C_f = small.tile([1, BL], f32)
        C_h = small.tile([1, BL], f32)
        sc1 = small.tile([BL, 1], f32)
        tblrow = small.tile([1, BL], f32)
        Ssts = [
            small.tile([112, w], bf16, name=f"Sst{i}")
            for i, (_, w) in enumerate(CH)
        ]
        Salts = [
            small.tile([112, w], bf16, name=f"Salt{i}")
            for i, (_, w) in enumerate(CH)
        ]

        p_sb = small.tile([T, BL], bf16)
        # explicit bias tiles for ACT (const-AP registry has no float biases)
        bz128 = small.tile([128, 1], f32)
        bdelta = small.tile([128, 1], f32)
        nc.gpsimd.memset(bz128[:], 0.0)
        nc.gpsimd.memset(bdelta[:], -float(delta))

        nc.sync.dma_start(out=c_sb[:], in_=c_d[:])
        nc.sync.dma_start(out=t19[:], in_=t19_d[:])
        nc.sync.dma_start(out=tr_sb[:], in_=tr_d[:])
        nc.sync.dma_start(out=trT_sb[:], in_=trT_d[:])
        nc.sync.dma_start(out=se_sb[:], in_=se_d[:])

        # blockdiag(E, E^T) weights + exp(start/end)
        nc.gpsimd.memset(lhs96[:], 0.0)
        nc.scalar.activation(
            out=lhs96[0:48, 0:48], in_=tr_sb[:], func=ACT.Exp, bias=bz128[0:48, :]
        )
        nc.scalar.activation(
            out=lhs96[64:112, 64:112], in_=trT_sb[:], func=ACT.Exp, bias=bz128[0:48, :]
        )
        nc.scalar.activation(
            out=es_ee[:], in_=se_sb[:], func=ACT.Exp, bias=bz128[0:48, :]
        )

        nc.gpsimd.memset(ones48[:], 1.0)
        nc.gpsimd.memset(onesel[:], 0.0)
        nc.gpsimd.memset(onesel[0:1, 0:64], 1.0)
        nc.gpsimd.memset(onesel[64:65, 64:112], 1.0)
        nc.gpsimd.memset(id1[:], 1.0)
        nc.gpsimd.memset(C_f[:], 0.0)
        nc.gpsimd.memset(C_h[:], 0.0)
        for Sst in Ssts:
            nc.gpsimd.memset(Sst[:], 0.0)

        # ---- emT load (cast f32->bf16 straight into tag-major layout) ----
        wins = [(0, 4), (4, 4), (8, 8), (16, 16)] + [
            (32 * k, 32) for k in range(1, 8)
        ]
        for b0w, nbw in wins:
            c0, c1 = b0w * BL, (b0w + nbw) * BL
            nc.gpsimd.dma_start(out=emT[:, c0:c1], in_=emT_d[:, c0:c1])
        # score-mask source: emissions (batch-major, cast to bf16) and the
        # one-hot mask, both behind the scan-critical emT windows on the Pool
        # queue; split in halves so the mask-multiply pipelines with the load
        H = GCOLS // 2
        nc.gpsimd.dma_start(out=emg[:, 0:H], in_=emg_d[:, 0:H])
        nc.gpsimd.dma_start(out=vhot[:, 0:H], in_=v_d[:, 0:H])
        nc.gpsimd.dma_start(out=emg[:, H:GCOLS], in_=emg_d[:, H:GCOLS])
        nc.gpsimd.dma_start(out=vhot[:, H:GCOLS], in_=v_d[:, H:GCOLS])

        # transition/start/end score: 19 PSUM-accumulated matmuls over the
        # host-built pair-count matrix (runs during the load phase, ahead of
        # the scan in the TensorE stream).
        tbl_ps = psN.tile([1, BL], f32, tag="n0", name="tblps")
        nc.scalar.copy(out=t19b[:], in_=t19[:])
        for k in range(NCH):
            nc.tensor.matmul(
                tbl_ps[:],
                lhsT=t19b[:, k : k + 1],
                rhs=c_sb[:, k * BL : (k + 1) * BL],
                start=(k == 0),
                stop=(k == NCH - 1),
            )
        nc.scalar.copy(out=tblrow[:], in_=tbl_ps[:])

        # ---- in-place exp per window: G = exp(em - delta), bf16 ----
        for b0w, nbw in wins:
            c0, c1 = b0w * BL, (b0w + nbw) * BL
            nc.scalar.activation(
                out=emT[:, c0:c1],
                in_=emT[:, c0:c1],
                func=ACT.Exp,
                bias=bdelta[0:112, :],
            )

        # emission-score mask-multiply on GPSIMD (hidden under the scan); the
        # summation runs on ACT once this finishes (emitted later).
        TTC = GCOLS // 8
        for i in range(8):
            c0, c1 = i * TTC, (i + 1) * TTC
            nc.gpsimd.tensor_tensor(
                out=vhot[:, c0:c1], in0=vhot[:, c0:c1], in1=emg[:, c0:c1],
                op=ALU.mult,
            )

        # ---- scan init: f_0 = exp(start)*G[0], h_511 = exp(end)*Gbwd[0] ----
        for Sst, (off, w) in zip(Ssts, CH):
            nc.vector.tensor_scalar_mul(
                Sst[0:48, :], emT[0:48, off : off + w], es_ee[:, 0:1]
            )
            nc.vector.tensor_scalar_mul(
                Sst[64:112, :], emT[64:112, off : off + w], es_ee[:, 1:2]
            )

        # ---- the 255 paired scan iterations, 3 interleaved batch chains.
        # DVE ops issued mid-scan must carry a PSUM operand: a DVE op with two
        # SBUF reads starves for the whole duration of any concurrent GPSIMD
        # elementwise op (shared SBUF read port).  So the reciprocal writes
        # its result to PSUM, and the per-renorm log-norms go to per-renorm
        # slots on ACT (summed on the DVE only at the very end).
        renorm_set = set(renorm_iters)
        cur = list(Ssts)
        alt = list(Salts)
        lnslots = []
        for s in range(1, NBLK):
            base = s * BL
            for ci, (off, w) in enumerate(CH):
                r_ps = psR.tile([112, w], f32, tag="r", name=f"r{ci}_{s}")
                # two concurrent 48x48 quadrant matmuls (short K=48 drains
                # overlap in the PE array); the 48:64 gap rows are never
                # touched by any matmul and may carry garbage harmlessly
                nc.tensor.matmul(
                    r_ps[0:48, :],
                    lhsT=lhs96[0:48, 0:48],
                    rhs=cur[ci][0:48, :],
                    start=True,
                    stop=True,
                )
                nc.tensor.matmul(
                    r_ps[64:112, :],
                    lhsT=lhs96[64:112, 64:112],
                    rhs=cur[ci][64:112, :],
                    start=True,
                    stop=True,
                )
                nc.vector.tensor_tensor(
                    out=cur[ci][:],
                    in0=r_ps[:],
                    in1=emT[:, base + off : base + off + w],
                    op=ALU.mult,
                )
            if s in renorm_set:
                lnf = small.tile([1, BL], f32, name=f"lnf_{s}")
                lnh = small.tile([1, BL], f32, name=f"lnh_{s}")
                lnslots.append((lnf, lnh))
                for ci, (off, w) in enumerate(CH):
                    nps = psN.tile(
                        [112, w], f32, tag=f"n{ci % 2}", name=f"n{ci}_{s}"
                    )
                    nc.tensor.matmul(
                        nps[0:48, :],
                        lhsT=onesel[0:48, 0:48],
                        rhs=cur[ci][0:48, :],
                        start=True,
                        stop=True,
                    )
                    nc.tensor.matmul(
                        nps[64:112, :],
                        lhsT=onesel[64:112, 64:112],
                        rhs=cur[ci][64:112, :],
                        start=True,
                        stop=True,
                    )
                    rn = psX.tile([112, w], f32, tag="rn", name=f"rn{ci}_{s}")
                    nc.vector.reciprocal(rn[:], nps[:])
                    nc.vector.tensor_tensor(
                        out=alt[ci][:], in0=cur[ci][:], in1=rn[:], op=ALU.mult
                    )
                    cur[ci], alt[ci] = alt[ci], cur[ci]
                    nc.scalar.activation(
                        out=lnf[:, off : off + w],
                        in_=nps[0:1, :],
                        func=ACT.Ln,
                        bias=bz128[0:1, :],
                    )
                    nc.scalar.activation(
                        out=lnh[:, off : off + w],
                        in_=nps[64:65, :],
                        func=ACT.Ln,
                        bias=bz128[0:1, :],
                    )

        # ---- finish: w_255 = E h_256; Z = sum_i f*w; partition = lnZ + C ----
        for ci, (off, w) in enumerate(CH):
            r_fin = psR.tile([112, w], f32, tag="r", name=f"rfin{ci}")
            nc.tensor.matmul(
                r_fin[64:112, :],
                lhsT=lhs96[64:112, 64:112],
                rhs=cur[ci][64:112, :],
                start=True,
                stop=True,
            )
            nc.vector.tensor_tensor(
                out=p_sb[:, off : off + w],
                in0=r_fin[64:112, :],
                in1=cur[ci][0:48, :],
                op=ALU.mult,
            )
        # ---- emission-score summation: chunked ACT accumulates (idle engine,
        # hidden under the scan); tile_wait_until keeps the scheduler from
        # hoisting them ahead of scan-critical ops in the engine FIFOs ----
        RCH = 4096
        nred = GCOLS // RCH
        sc_part = small.tile([BL, nred], f32)
        with tc.tile_wait_until(0.12):
            for i in range(nred):
                nc.scalar.activation(
                    out=vhot[:, i * RCH : (i + 1) * RCH],
                    in_=vhot[:, i * RCH : (i + 1) * RCH],
                    func=ACT.Copy,
                    accum_out=sc_part[:, i : i + 1],
                )
            nc.vector.tensor_reduce(
                out=sc1[:], in_=sc_part[:], axis=mybir.AxisListType.X, op=ALU.add
            )
            # sum the per-renorm log-norms and fold the (negated) table
            # score into C_f; Pool is idle by now so all-SBUF ops are safe
            for lnf, lnh in lnslots:
                nc.vector.tensor_add(C_f[:], C_f[:], lnf[:])
                nc.vector.tensor_add(C_h[:], C_h[:], lnh[:])
            nc.vector.tensor_sub(C_f[:], C_f[:], tblrow[:])

        z_ps = ps1.tile([BL, 1], f32)
        nc.tensor.matmul(z_ps[:], lhsT=p_sb[:], rhs=ones48[:], start=True, stop=True)
        lnz = small.tile([BL, 1], f32)
        nc.scalar.activation(out=lnz[:], in_=z_ps[:], func=ACT.Ln, bias=bz128[:])

        cT = ps1.tile([BL, 2], f32)
        nc.tensor.transpose(cT[:, 0:1], in_=C_f[:], identity=id1[:])
        nc.tensor.transpose(cT[:, 1:2], in_=C_h[:], identity=id1[:])

        nllv = small.tile([BL, 1], f32)
        nc.vector.tensor_add(nllv[:], lnz[:], cT[:, 0:1])
        nc.vector.tensor_add(nllv[:], nllv[:], cT[:, 1:2])
        nc.vector.tensor_sub(nllv[:], nllv[:], sc1[:])
        nc.vector.tensor_scalar_add(nllv[:], nllv[:], float(S * delta))
        nc.sync.dma_start(out=out_d[:], in_=nllv[:])

    nc.compile()
    return nc


def _host_inputs(emissions, tags, transitions, start_transitions, end_transitions):
    """Per-core input dicts (pure data movement / index prep on host: layout
    transposes of the input tensors plus one-hot/count encodings of the tag
    indices; no arithmetic ever touches the float inputs)."""
    import ml_dtypes

    em = np.ascontiguousarray(np.asarray(emissions, dtype=np.float32))
    tg = np.asarray(tags, dtype=np.int64)
    tr = np.ascontiguousarray(np.asarray(transitions, dtype=np.float32))
    st = np.asarray(start_transitions, dtype=np.float32)
    en = np.asarray(end_transitions, dtype=np.float32)

    # tag-major paired layout with a zero gap at rows 48:64: [112, NBLK, B]
    emT_full = np.zeros((112, NBLK, B), dtype=np.float32)
    emT_full[0:48] = em[0:NBLK].transpose(2, 0, 1)
    emT_full[64:112] = em[S - 1 : NBLK - 1 : -1].transpose(2, 0, 1)
    # batch-major layout for the emission-score mask-sum: [B, S*T]
    emg_full = em.transpose(1, 0, 2).reshape(B, GCOLS)

    trT = np.ascontiguousarray(tr.T)
    se = np.ascontiguousarray(np.stack([st, en], axis=1))

    # one-hot of the gold tag per (b, s): V[b, T*s + tag[s,b]] = 1
    pos = (np.arange(B)[None, :] * GCOLS + T * np.arange(S)[:, None] + tg).ravel()
    v_full = np.zeros(B * GCOLS, dtype=ml_dtypes.bfloat16)
    v_full[pos] = 1
    v_full = v_full.reshape(B, GCOLS)

    # per-batch transition-pair counts + start/end one-hots: C_ext[2432, B]
    q = T * tg[:-1] + tg[1:]                         # (S-1, B)
    bcol = np.arange(B)
    c_ext = np.bincount(
        (q * B + bcol[None, :]).ravel(), minlength=T * T * B
    ).reshape(T * T, B).astype(np.float32)
    c_ext = np.concatenate([c_ext, np.zeros((CC - T * T, B), np.float32)], axis=0)
    c_ext[T * T + tg[0], bcol] += 1.0                # start one-hot
    c_ext[T * T + T + tg[S - 1], bcol] += 1.0        # end one-hot
    t_ext = np.zeros(CC, np.float32)
    t_ext[0 : T * T] = tr.reshape(-1)
    t_ext[T * T : T * T + T] = st
    t_ext[T * T + T : T * T + 2 * T] = en
    t19 = np.ascontiguousarray(t_ext.reshape(NCH, 128).T)      # [128, NCH]

    in_maps = []
    for c in range(NCORES):
        b0, b1 = c * BL, (c + 1) * BL
        c_core = np.ascontiguousarray(
            c_ext[:, b0:b1].reshape(NCH, 128, BL).transpose(1, 0, 2).reshape(128, CC)
        ).astype(ml_dtypes.bfloat16)
        in_maps.append(
            {
                "emT": np.ascontiguousarray(
                    emT_full[:, :, b0:b1].reshape(112, EMCOLS)
                ),
                "emg": np.ascontiguousarray(emg_full[b0:b1]),
                "vhot": np.ascontiguousarray(v_full[b0:b1]),
                "cmat": c_core,
                "t19": t19,
                "transitions": tr,
                "transitionsT": trT,
                "startend": se,
            }
        )
    return in_maps


def kernel(emissions, tags, mask, transitions, start_transitions, end_transitions):
    delta, R = _estimate_delta(
        np.asarray(emissions, np.float32),
        np.asarray(transitions, np.float32),
        np.asarray(start_transitions, np.float32),
    )
    renorm_iters = list(range(R, NBLK, R))
    nc = _build(delta, renorm_iters)
    in_maps = _host_inputs(
        emissions, tags, transitions, start_transitions, end_transitions
    )
    res = run_bass_kernel_spmd(nc, in_maps, core_ids=list(range(NCORES)))
    _LAST["results"] = res
    _LAST["delta"] = delta
    _LAST["R"] = R
    total = 0.0
    for c in range(NCORES):
        total += float(res.results[c]["nll"].astype(np.float64).sum())
    return np.asarray(total, dtype=np.float32)


# revision 23
# speedup vs baseline: 1.1084x; 1.1084x over previous
"""CRF negative log-likelihood (sum) on 8 Trainium2 NeuronCores.

Strategy (per core, batch-sharded 1024 -> 8 x 128):
  partition function: linear-space bidirectional scan.
    alpha recurrence rewritten as f_s = G[s] * (E^T f_{s-1}),
    beta  recurrence rewritten as h_t = G[t] * (E h_{t+1}),
    with E = exp(transitions), G[t] = exp(em[t] - delta)  (delta = host-estimated
    mean log-growth, for fp range safety; periodic renormalization adds the
    removed log-factors back).
    Both chains run in the SAME instructions: state S = [f ; h] (96 x batch),
    one blockdiag(E, E^T) matmul + one elementwise multiply per paired step.
    The 128-batch shard is further split into THREE independent ~43-wide
    chains so the serial matmul->multiply dependency chains interleave on
    TensorE/DVE (each engine works on one chain while the others' results are
    in flight), cutting the per-step critical path well below the fused form.
    Z_b = sum_i f_255[i,b] * (E h_256)[i,b];  logZ + 512*delta + C = partition.
  score: no gathers.  The emission part sum_s em[s,b,tag] is a masked sum
    of the batch-major emissions against a host-built one-hot V (0/1 bf16):
    the multiply runs on GPSIMD and the summation on ACT accumulate chunks,
    both hidden under the scan (scheduled late via tile_wait_until).  The
    transition/start/end part is sum_ij C_b[i,j]*T[i,j] with C_b a host-built
    per-batch pair-count matrix (pure tag statistics): 19 PSUM-accumulated
    matmuls during the load phase.  Mid-scan DVE ops always carry a PSUM
    operand: a DVE op with two SBUF reads starves for the duration of any
    concurrent GPSIMD elementwise op (shared SBUF read port).
  mask input is all-ones per the problem spec and is not used.

Layouts (host prepares pure reorderings of the inputs, no arithmetic):
  emT_d [112, 256*128]: tag-major paired emissions; emT[j, k*128+b] = em[k,b,j]
    for j<48, em[511-k,b,j-64] for 64<=j<112, zeros in the 48:64 gap (the gap
    keeps every compute-engine partition base 32-aligned).  DMA'd straight
    into SBUF with an f32->bf16 casting DMA (no on-device transposes), exp'd
    in place.
  emg_d [128, 512*48]: batch-major emissions for the score mask-sum.
"""

import os
from contextlib import ExitStack

import numpy as np

import concourse.bass as bass
import concourse.bacc as bacc
import concourse.mybir as mybir
import concourse.tile as tile
from concourse.bass_utils import run_bass_kernel_spmd

S, B, T = 512, 1024, 48
NCORES = 8
BL = B // NCORES          # 128 batch per core
CH = ((0, 44), (44, 42), (86, 42))  # (offset, width) of the 3 batch chains
NBLK = S // 2             # 256 paired blocks
EMCOLS = NBLK * BL        # 32768 cols in emT
NWIN = 16                 # load/exp windows
WBLK = NBLK // NWIN       # 32 blocks per window
GCOLS = S * T             # 24576 score-mask cols for emissions
NCH = 19                  # 19 x 128 = 2432 padded count rows (2400 used)
CC = NCH * 128            # 2432

f32 = mybir.dt.float32
bf16 = mybir.dt.bfloat16
u16 = mybir.dt.uint16
ALU = mybir.AluOpType
ACT = mybir.ActivationFunctionType

_LAST = {}


def _estimate_delta(em, tr, st, nsamp=16):
    """Exact log-space forward scan on a few batch columns: mean per-step
    log-growth (delta) and renorm period R keeping |log f| bounded."""
    e = np.asarray(em[:, :nsamp, :], dtype=np.float64)
    trr = np.asarray(tr, dtype=np.float64)
    a = np.asarray(st, dtype=np.float64)[None, :] + e[0]
    means = [a.mean(axis=1)]
    for t in range(1, S):
        z = a[:, :, None] + trr[None, :, :]
        m = z.max(axis=1)
        a = e[t] + m + np.log(np.exp(z - m[:, None, :]).sum(axis=1))
        means.append(a.mean(axis=1))
    meanarr = np.stack(means)               # (S, nsamp)
    inc = np.diff(meanarr, axis=0)          # (S-1, nsamp)
    delta = float(inc.mean())
    dev = np.concatenate(
        [np.zeros((1, nsamp)), np.cumsum(inc - delta, axis=0)], axis=0
    )
    R = 16
    for cand in (64, 32, 16):
        wmax = 0.0
        for w0 in range(0, S - 1, cand):
            seg = dev[w0 : w0 + cand + 1] - dev[w0]
            wmax = max(wmax, float(np.abs(seg).max()))
        if 1.5 * wmax + 16.0 < 70.0:
            R = cand
            break
    return delta, R


def _build(delta, renorm_iters):
    nc = bacc.Bacc("TRN2", target_bir_lowering=False, debug=False)
    emT_d = nc.dram_tensor("emT", [112, EMCOLS], f32, kind="ExternalInput")
    emg_d = nc.dram_tensor("emg", [BL, GCOLS], f32, kind="ExternalInput")
    v_d = nc.dram_tensor("vhot", [BL, GCOLS], bf16, kind="ExternalInput")
    c_d = nc.dram_tensor("cmat", [128, CC], bf16, kind="ExternalInput")
    t19_d = nc.dram_tensor("t19", [128, NCH], f32, kind="ExternalInput")
    tr_d = nc.dram_tensor("transitions", [T, T], f32, kind="ExternalInput")
    trT_d = nc.dram_tensor("transitionsT", [T, T], f32, kind="ExternalInput")
    se_d = nc.dram_tensor("startend", [T, 2], f32, kind="ExternalInput")
    out_d = nc.dram_tensor("nll", [BL, 1], f32, kind="ExternalOutput")

    with tile.TileContext(nc) as tc, ExitStack() as ctx:
        big = ctx.enter_context(tc.tile_pool(name="big", bufs=1))
        small = ctx.enter_context(tc.tile_pool(name="small", bufs=1))
        rn_pool = ctx.enter_context(tc.tile_pool(name="rnp", bufs=2))
        psR = ctx.enter_context(tc.tile_pool(name="psR", bufs=3, space="PSUM"))
        psN = ctx.enter_context(tc.tile_pool(name="psN", bufs=1, space="PSUM"))
        psX = ctx.enter_context(tc.tile_pool(name="psX", bufs=1, space="PSUM"))
        ps1 = ctx.enter_context(tc.tile_pool(name="ps1", bufs=1, space="PSUM"))

        emT = big.tile([112, EMCOLS], bf16)
        emg = big.tile([BL, GCOLS], bf16)
        vhot = big.tile([BL, GCOLS], bf16)
        c_sb = big.tile([128, CC], bf16)

        # ---- small constant tiles ----
        tr_sb = small.tile([T, T], f32)
        trT_sb = small.tile([T, T], f32)
        se_sb = small.tile([T, 2], f32)
        es_ee = small.tile([T, 2], f32)
        t19 = small.tile([128, NCH], f32)
        t19b = small.tile([128, NCH], bf16)
        lhs96 = small.tile([112, 112], bf16)
        onesel = small.tile([112, 112], bf16)
        ones48 = small.tile([T, 1], bf16)
        id1 = small.tile([1, 1], f32)
        C_f = small.tile([1, BL], f32)
        C_h = small.tile([1, BL], f32)
        sc1 = small.tile([BL, 1], f32)
        tblrow = small.tile([1, BL], f32)
        Ssts = [
            small.tile([112, w], bf16, name=f"Sst{i}")
            for i, (_, w) in enumerate(CH)
        ]
        Salts = [
            small.tile([112, w], bf16, name=f"Salt{i}")
            for i, (_, w) in enumerate(CH)
        ]

        p_sb = small.tile([T, BL], bf16)
        # explicit bias tiles for ACT (const-AP registry has no float biases)
        bz128 = small.tile([128, 1], f32)
        bdelta = small.tile([128, 1], f32)
        nc.gpsimd.memset(bz128[:], 0.0)
        nc.gpsimd.memset(bdelta[:], -float(delta))

        nc.sync.dma_start(out=c_sb[:], in_=c_d[:])
        nc.sync.dma_start(out=t19[:], in_=t19_d[:])
        nc.sync.dma_start(out=tr_sb[:], in_=tr_d[:])
        nc.sync.dma_start(out=trT_sb[:], in_=trT_d[:])
        nc.sync.dma_start(out=se_sb[:], in_=se_d[:])

        # blockdiag(E, E^T) weights + exp(start/end)
        nc.gpsimd.memset(lhs96[:], 0.0)
        nc.scalar.activation(
            out=lhs96[0:48, 0:48], in_=tr_sb[:], func=ACT.Exp, bias=bz128[0:48, :]
        )
        nc.scalar.activation(
            out=lhs96[64:112, 64:112], in_=trT_sb[:], func=ACT.Exp, bias=bz128[0:48, :]
        )
        nc.scalar.activation(
            out=es_ee[:], in_=se_sb[:], func=ACT.Exp, bias=bz128[0:48, :]
        )

        nc.gpsimd.memset(ones48[:], 1.0)
        nc.gpsimd.memset(onesel[:], 0.0)
        nc.gpsimd.memset(onesel[0:1, 0:64], 1.0)
        nc.gpsimd.memset(onesel[64:65, 64:112], 1.0)
        nc.gpsimd.memset(id1[:], 1.0)
        nc.gpsimd.memset(C_f[:], 0.0)
        nc.gpsimd.memset(C_h[:], 0.0)
        for Sst in Ssts:
            nc.gpsimd.memset(Sst[:], 0.0)

        # ---- emT load (cast f32->bf16 straight into tag-major layout) ----
        wins = [(0, 4), (4, 4), (8, 8), (16, 16)] + [
            (32 * k, 32) for k in range(1, 8)
        ]
        for b0w, nbw in wins:
            c0, c1 = b0w * BL, (b0w + nbw) * BL
            nc.gpsimd.dma_start(out=emT[:, c0:c1], in_=emT_d[:, c0:c1])
        # score-mask source: emissions (batch-major, cast to bf16) and the
        # one-hot mask, both behind the scan-critical emT windows on the Pool
        # queue; split in halves so the mask-multiply pipelines with the load
        H = GCOLS // 2
        nc.gpsimd.dma_start(out=emg[:, 0:H], in_=emg_d[:, 0:H])
        nc.gpsimd.dma_start(out=vhot[:, 0:H], in_=v_d[:, 0:H])
        nc.gpsimd.dma_start(out=emg[:, H:GCOLS], in_=emg_d[:, H:GCOLS])
        nc.gpsimd.dma_start(out=vhot[:, H:GCOLS], in_=v_d[:, H:GCOLS])

        # transition/start/end score: 19 PSUM-accumulated matmuls over the
        # host-built pair-count matrix (runs during the load phase, ahead of
        # the scan in the TensorE stream).
        tbl_ps = psN.tile([1, BL], f32, tag="n0", name="tblps")
        nc.scalar.copy(out=t19b[:], in_=t19[:])
        for k in range(NCH):
            nc.tensor.matmul(
                tbl_ps[:],
                lhsT=t19b[:, k : k + 1],
                rhs=c_sb[:, k * BL : (k + 1) * BL],
                start=(k == 0),
                stop=(k == NCH - 1),
            )
        nc.scalar.copy(out=tblrow[:], in_=tbl_ps[:])

        # ---- in-place exp per window: G = exp(em - delta), bf16 ----
        for b0w, nbw in wins:
            c0, c1 = b0w * BL, (b0w + nbw) * BL
            nc.scalar.activation(
                out=emT[:, c0:c1],
                in_=emT[:, c0:c1],
                func=ACT.Exp,
                bias=bdelta[0:112, :],
            )

        # emission-score mask-multiply on GPSIMD (hidden under the scan); the
        # summation runs on ACT once this finishes (emitted later).
        TTC = GCOLS // 8
        for i in range(8):
            c0, c1 = i * TTC, (i + 1) * TTC
            nc.gpsimd.tensor_tensor(
                out=vhot[:, c0:c1], in0=vhot[:, c0:c1], in1=emg[:, c0:c1],
                op=ALU.mult,
            )

        # ---- scan init: f_0 = exp(start)*G[0], h_511 = exp(end)*Gbwd[0] ----
        for Sst, (off, w) in zip(Ssts, CH):
            nc.vector.tensor_scalar_mul(
                Sst[0:48, :], emT[0:48, off : off + w], es_ee[:, 0:1]
            )
            nc.vector.tensor_scalar_mul(
                Sst[64:112, :], emT[64:112, off : off + w], es_ee[:, 1:2]
            )

        # ---- the 255 paired scan iterations, 3 interleaved batch chains.
        # DVE ops issued mid-scan must carry a PSUM operand: a DVE op with two
        # SBUF reads starves for the whole duration of any concurrent GPSIMD
        # elementwise op (shared SBUF read port).  So the reciprocal writes
        # its result to PSUM, and the per-renorm log-norms go to per-renorm
        # slots on ACT (summed on the DVE only at the very end).
        renorm_set = set(renorm_iters)
        cur = list(Ssts)
        alt = list(Salts)
        lnslots = []
        for s in range(1, NBLK):
            base = s * BL
            for ci, (off, w) in enumerate(CH):
                r_ps = psR.tile([112, w], f32, tag="r", name=f"r{ci}_{s}")
                nc.tensor.matmul(
                    r_ps[:], lhsT=lhs96[:], rhs=cur[ci][:], start=True, stop=True
                )
                nc.vector.tensor_tensor(
                    out=cur[ci][:],
                    in0=r_ps[:],
                    in1=emT[:, base + off : base + off + w],
                    op=ALU.mult,
                )
            if s in renorm_set:
                lnf = small.tile([1, BL], f32, name=f"lnf_{s}")
                lnh = small.tile([1, BL], f32, name=f"lnh_{s}")
                lnslots.append((lnf, lnh))
                for ci, (off, w) in enumerate(CH):
                    nps = psN.tile(
                        [112, w], f32, tag=f"n{ci % 2}", name=f"n{ci}_{s}"
                    )
                    nc.tensor.matmul(
                        nps[:], lhsT=onesel[:], rhs=cur[ci][:], start=True, stop=True
                    )
                    rn = psX.tile([112, w], f32, tag="rn", name=f"rn{ci}_{s}")
                    nc.vector.reciprocal(rn[:], nps[:])
                    nc.vector.tensor_tensor(
                        out=alt[ci][:], in0=cur[ci][:], in1=rn[:], op=ALU.mult
                    )
                    cur[ci], alt[ci] = alt[ci], cur[ci]
                    nc.scalar.activation(
                        out=lnf[:, off : off + w],
                        in_=nps[0:1, :],
                        func=ACT.Ln,
                        bias=bz128[0:1, :],
                    )
                    nc.scalar.activation(
                        out=lnh[:, off : off + w],
                        in_=nps[64:65, :],
                        func=ACT.Ln,
                        bias=bz128[0:1, :],
                    )

        # ---- finish: w_255 = E h_256; Z = sum_i f*w; partition = lnZ + C ----
        for ci, (off, w) in enumerate(CH):
            r_fin = psR.tile([112, w], f32, tag="r", name=f"rfin{ci}")
            nc.tensor.matmul(
                r_fin[:], lhsT=lhs96[:], rhs=cur[ci][:], start=True, stop=True
            )
            nc.vector.tensor_tensor(
                out=p_sb[:, off : off + w],
                in0=r_fin[64:112, :],
                in1=cur[ci][0:48, :],
                op=ALU.mult,
            )
        # ---- emission-score summation: chunked ACT accumulates (idle engine,
        # hidden under the scan); tile_wait_until keeps the scheduler from
        # hoisting them ahead of scan-critical ops in the engine FIFOs ----
        RCH = 4096
        nred = GCOLS // RCH
        sc_part = small.tile([BL, nred], f32)
        with tc.tile_wait_until(0.12):
            for i in range(nred):
                nc.scalar.activation(
                    out=vhot[:, i * RCH : (i + 1) * RCH],
                    in_=vhot[:, i * RCH : (i + 1) * RCH],
                    func=ACT.Copy,
                    accum_out=sc_part[:, i : i + 1],
                )
            nc.vector.tensor_reduce(
                out=sc1[:], in_=sc_part[:], axis=mybir.AxisListType.X, op=ALU.add
            )
            # sum the per-renorm log-norms and fold the (negated) table
            # score into C_f; Pool is idle by now so all-SBUF ops are safe
            for lnf, lnh in lnslots:
                nc.vector.tensor_add(C_f[:], C_f[:], lnf[:])
                nc.vector.tensor_add(C_h[:], C_h[:], lnh[:])
            nc.vector.tensor_sub(C_f[:], C_f[:], tblrow[:])

        z_ps = ps1.tile([BL, 1], f32)
        nc.tensor.matmul(z_ps[:], lhsT=p_sb[:], rhs=ones48[:], start=True, stop=True)
        lnz = small.tile([BL, 1], f32)
        nc.scalar.activation(out=lnz[:], in_=z_ps[:], func=ACT.Ln, bias=bz128[:])

        cT = ps1.tile([BL, 2], f32)
        nc.tensor.transpose(cT[:, 0:1], in_=C_f[:], identity=id1[:])
        nc.tensor.transpose(cT[:, 1:2], in_=C_h[:], identity=id1[:])

        nllv = small.tile([BL, 1], f32)
        nc.vector.tensor_add(nllv[:], lnz[:], cT[:, 0:1])
        nc.vector.tensor_add(nllv[:], nllv[:], cT[:, 1:2])
        nc.vector.tensor_sub(nllv[:], nllv[:], sc1[:])
        nc.vector.tensor_scalar_add(nllv[:], nllv[:], float(S * delta))
        nc.sync.dma_start(out=out_d[:], in_=nllv[:])

    nc.compile()
    return nc


def _host_inputs(emissions, tags, transitions, start_transitions, end_transitions):
    """Per-core input dicts (pure data movement / index prep on host: layout
    transposes of the input tensors plus one-hot/count encodings of the tag
    indices; no arithmetic ever touches the float inputs)."""
    import ml_dtypes

    em = np.ascontiguousarray(np.asarray(emissions, dtype=np.float32))
    tg = np.asarray(tags, dtype=np.int64)
    tr = np.ascontiguousarray(np.asarray(transitions, dtype=np.float32))
    st = np.asarray(start_transitions, dtype=np.float32)
    en = np.asarray(end_transitions, dtype=np.float32)

    # tag-major paired layout with a zero gap at rows 48:64: [112, NBLK, B]
    emT_full = np.zeros((112, NBLK, B), dtype=np.float32)
    emT_full[0:48] = em[0:NBLK].transpose(2, 0, 1)
    emT_full[64:112] = em[S - 1 : NBLK - 1 : -1].transpose(2, 0, 1)
    # batch-major layout for the emission-score mask-sum: [B, S*T]
    emg_full = em.transpose(1, 0, 2).reshape(B, GCOLS)

    trT = np.ascontiguousarray(tr.T)
    se = np.ascontiguousarray(np.stack([st, en], axis=1))

    # one-hot of the gold tag per (b, s): V[b, T*s + tag[s,b]] = 1
    pos = (np.arange(B)[None, :] * GCOLS + T * np.arange(S)[:, None] + tg).ravel()
    v_full = np.zeros(B * GCOLS, dtype=ml_dtypes.bfloat16)
    v_full[pos] = 1
    v_full = v_full.reshape(B, GCOLS)

    # per-batch transition-pair counts + start/end one-hots: C_ext[2432, B]
    q = T * tg[:-1] + tg[1:]                         # (S-1, B)
    bcol = np.arange(B)
    c_ext = np.bincount(
        (q * B + bcol[None, :]).ravel(), minlength=T * T * B
    ).reshape(T * T, B).astype(np.float32)
    c_ext = np.concatenate([c_ext, np.zeros((CC - T * T, B), np.float32)], axis=0)
    c_ext[T * T + tg[0], bcol] += 1.0                # start one-hot
    c_ext[T * T + T + tg[S - 1], bcol] += 1.0        # end one-hot
    t_ext = np.zeros(CC, np.float32)
    t_ext[0 : T * T] = tr.reshape(-1)
    t_ext[T * T : T * T + T] = st
    t_ext[T * T + T : T * T + 2 * T] = en
    t19 = np.ascontiguousarray(t_ext.reshape(NCH, 128).T)      # [128, NCH]

    in_maps = []
    for c in range(NCORES):
        b0, b1 = c * BL, (c + 1) * BL
        c_core = np.ascontiguousarray(
            c_ext[:, b0:b1].reshape(NCH, 128, BL).transpose(1, 0, 2).reshape(128, CC)
        ).astype(ml_dtypes.bfloat16)
        in_maps.append(
            {
                "emT": np.ascontiguousarray(
                    emT_full[:, :, b0:b1].reshape(112, EMCOLS)
                ),
                "emg": np.ascontiguousarray(emg_full[b0:b1]),
                "vhot": np.ascontiguousarray(v_full[b0:b1]),
                "cmat": c_core,
                "t19": t19,
                "transitions": tr,
                "transitionsT": trT,
                "startend": se,
            }
        )
    return in_maps


def kernel(emissions, tags, mask, transitions, start_transitions, end_transitions):
    delta, R = _estimate_delta(
        np.asarray(emissions, np.float32),
        np.asarray(transitions, np.float32),
        np.asarray(start_transitions, np.float32),
    )
    renorm_iters = list(range(R, NBLK, R))
    nc = _build(delta, renorm_iters)
    in_maps = _host_inputs(
        emissions, tags, transitions, start_transitions, end_transitions
    )
    res = run_bass_kernel_spmd(nc, in_maps, core_ids=list(range(NCORES)))
    _LAST["results"] = res
    _LAST["delta"] = delta
    _LAST["R"] = R
    total = 0.0
    for c in range(NCORES):
        total += float(res.results[c]["nll"].astype(np.float64).sum())
    return np.asarray(total, dtype=np.float32)
